# revision 1
# baseline (speedup 1.0000x reference)
"""Trainium2 Bass kernel for modulated deformable attention (deform_conv2d v2).

Sharding: data-parallel over batch B=8, one image per NeuronCore.

Device algorithm per core (v2):
  - offset/attn convs on PE as 9 shifted-AP matmuls accumulating in PSUM.
  - softmax over taps via PE selector matmuls + DVE reciprocal.
  - bilinear sampling expanded over a dense 5x5 integer shift window around
    each tap: samp = sum_{rr,ss} hat(offy-rr)*hat(offx-ss)*x_shift, with
    hat(t)=relu(1-|t|) the exact bilinear kernel (offsets beyond +-2 are
    truncated; empirical max |off| = 2.7, ~2e-6 of sites affected).
  - modulation maps M[(g,k),(rr,ss),p] = attn*hat*hat built on ACT/DVE at
    (g,k) partition rows, replicated across each group's 32 channel lanes
    via a DRAM round-trip DMA.
  - per-shift products on DVE (bf16, parity-aligned via a 1-px-shifted
    image copy); the 25-shift accumulation runs on PE as identity-matmul
    accumulation into PSUM (u in fp32), then one DVE copy to bf16.
  - final contraction over (g,c,k)=2304 on PE in 18 accumulation chunks.
"""
import numpy as np
import ml_dtypes

G, KK, Kk = 8, 9, 3
Cg, C, O = 32, 256, 256
H = W = 64
HW = H * W
PAD = 4
Hp = Wp = H + 2 * PAD  # 72
NPIX = Hp * Wp  # 5184
RR = SS = 5  # shift window [-2..2] around each tap
NSTRIPE = 8
SH = H // NSTRIPE  # 8 dst rows per stripe
SDST = SH * W  # 512 dst pixels per stripe
MQ = RR * SS * SDST  # map elements per (g,k) per stripe

BF16 = ml_dtypes.bfloat16

_COMPILED = {}


def _build_kernel():
    import concourse.bass as bass
    import concourse.bacc as bacc
    import concourse.tile as tile
    import concourse.mybir as mybir

    f32 = mybir.dt.float32
    bf16 = mybir.dt.bfloat16
    AF = mybir.ActivationFunctionType

    nc = bacc.Bacc("TRN2", target_bir_lowering=False, num_devices=8)

    xq_d = nc.dram_tensor("xq", [2, 128, NPIX], bf16, kind="ExternalInput")
    xqo_d = nc.dram_tensor("xqo", [2, 128, NPIX], bf16, kind="ExternalInput")
    wmat_d = nc.dram_tensor("wmat", [128, 9 * 2 * 216], bf16, kind="ExternalInput")
    wt2_d = nc.dram_tensor("wt2", [128, 2 * KK * O], bf16, kind="ExternalInput")
    sel_d = nc.dram_tensor("sel", [72, 8], f32, kind="ExternalInput")
    rep_d = nc.dram_tensor("rep", [8, 72], f32, kind="ExternalInput")
    bias_d = nc.dram_tensor("biasc", [128, 9], f32, kind="ExternalInput")
    bout_d = nc.dram_tensor("bout", [128, 2], f32, kind="ExternalInput")
    ident_d = nc.dram_tensor("ident", [128, 128], bf16, kind="ExternalInput")
    out_d = nc.dram_tensor("out", [O, HW], f32, kind="ExternalOutput")
    # DRAM scratch
    m_scr = [nc.dram_tensor(f"mscr{i}", [72, MQ], bf16) for i in range(2)]
    offy_d = nc.dram_tensor("offy_scr", [72, HW], bf16)
    offx_d = nc.dram_tensor("offx_scr", [72, HW], bf16)
    aw_d = nc.dram_tensor("aw_scr", [72, HW], bf16)

    def win(t, anchor, dims):
        ap = t[:]
        return bass.AP(ap.tensor, ap.offset + anchor,
                       [[ap.ap[0][0], ap.ap[0][1]]] + [list(d) for d in dims])

    with tile.TileContext(nc) as tc:
        with tc.tile_pool(name="io", bufs=1) as io_pool:
            dma = nc.sync.dma_start

            xq = [io_pool.tile([128, NPIX], bf16, tag=f"xq{q}", name=f"xq{q}")
                  for q in range(2)]
            xqo = [io_pool.tile([128, NPIX], bf16, tag=f"xqo{q}", name=f"xqo{q}")
                   for q in range(2)]
            for q in range(2):
                dma(xq[q][:], xq_d[q])
                dma(xqo[q][:], xqo_d[q])
            wt2 = io_pool.tile([128, 2 * KK * O], bf16)
            dma(wt2[:], wt2_d[:])
            sel = io_pool.tile([72, 8], f32)
            dma(sel[:], sel_d[:])
            rep = io_pool.tile([8, 72], f32)
            dma(rep[:], rep_d[:])
            biasc = io_pool.tile([128, 9], f32)
            dma(biasc[:], bias_d[:])
            bout = io_pool.tile([128, 2], f32)
            dma(bout[:], bout_d[:])
            ident = io_pool.tile([128, 128], bf16)
            dma(ident[:], ident_d[:])

            # ---- Phase B: convs -> offy/offx/aw, spilled to DRAM ----
            NT = 512
            with (
                tc.tile_pool(name="wm", bufs=1) as wm_pool,
                tc.tile_pool(name="cpsum", bufs=2,
                             space=bass.MemorySpace.PSUM) as cpsum,
                tc.tile_pool(name="cwork", bufs=2) as cwork,
            ):
                wmat = wm_pool.tile([128, 9 * 2 * 216], bf16)
                dma(wmat[:], wmat_d[:])

                def wmat_ap(s, q, m0, m1):
                    base = (s * 2 + q) * 216
                    return wmat[:, base + m0: base + m1]

                for nt in range(HW // NT):
                    h0 = nt * (NT // W)
                    ps_y = cpsum.tile([72, NT], f32, tag="ps_y")
                    ps_x = cpsum.tile([72, NT], f32, tag="ps_x")
                    ps_a = cpsum.tile([72, NT], f32, tag="ps_a")
                    first = True
                    for dy in range(3):
                        for dx in range(3):
                            s = dy * 3 + dx
                            for q in range(2):
                                anchor = ((h0 + PAD + dy - 1) * Wp
                                          + (PAD + dx - 1))
                                rhs = win(xq[q], anchor, [[Wp, NT // W], [1, W]])
                                last = (s == 8) and (q == 1)
                                nc.tensor.matmul(ps_y[:], wmat_ap(s, q, 0, 72),
                                                 rhs, start=first, stop=last)
                                nc.tensor.matmul(ps_x[:], wmat_ap(s, q, 72, 144),
                                                 rhs, start=first, stop=last)
                                nc.tensor.matmul(ps_a[:], wmat_ap(s, q, 144, 216),
                                                 rhs, start=first, stop=last)
                                first = False
                    oy = cwork.tile([72, NT], bf16, tag="oy")
                    ox = cwork.tile([72, NT], bf16, tag="ox")
                    nc.scalar.activation(oy[:], ps_y[:], AF.Identity,
                                         bias=biasc[0:72, 0:1])
                    nc.scalar.activation(ox[:], ps_x[:], AF.Identity,
                                         bias=biasc[0:72, 1:2])
                    att_e = cwork.tile([72, NT], f32, tag="att_e")
                    nc.scalar.activation(att_e[:], ps_a[:], AF.Exp,
                                         bias=biasc[0:72, 2:3])
                    ps_s = cpsum.tile([8, NT], f32, tag="ps_s", bufs=1)
                    nc.tensor.matmul(ps_s[:], sel[:], att_e[:],
                                     start=True, stop=True)
                    rcp = cwork.tile([8, NT], f32, tag="rcp")
                    nc.vector.reciprocal(rcp[:], ps_s[:])
                    ps_r = cpsum.tile([72, NT], f32, tag="ps_r", bufs=1)
                    nc.tensor.matmul(ps_r[:], rep[:], rcp[:],
                                     start=True, stop=True)
                    awt = cwork.tile([72, NT], bf16, tag="awt")
                    nc.vector.tensor_mul(awt[:], att_e[:], ps_r[:])
                    sl = slice(nt * NT, (nt + 1) * NT)
                    dma(offy_d[:, sl], oy[:])
                    dma(offx_d[:, sl], ox[:])
                    dma(aw_d[:, sl], awt[:])

            # ---- Phase D: stripes ----
            with (
                tc.tile_pool(name="hat", bufs=1) as hat_pool,
                tc.tile_pool(name="mfull", bufs=1) as mfull_pool,
                tc.tile_pool(name="rep2", bufs=2) as rep_pool,
                tc.tile_pool(name="u", bufs=1) as u_pool,
                tc.tile_pool(name="dpsum", bufs=2,
                             space=bass.MemorySpace.PSUM) as dpsum,
                tc.tile_pool(name="dwork", bufs=2) as dwork,
            ):
                for st in range(NSTRIPE):
                    h0 = st * SH
                    dsl = slice(st * SDST, (st + 1) * SDST)
                    mscr = m_scr[st % 2]

                    oy_s = hat_pool.tile([72, SDST], bf16, tag="oy_s")
                    ox_s = hat_pool.tile([72, SDST], bf16, tag="ox_s")
                    aw_s = hat_pool.tile([72, SDST], bf16, tag="aw_s")
                    dma(oy_s[:], offy_d[:, dsl])
                    dma(ox_s[:], offx_d[:, dsl])
                    dma(aw_s[:], aw_d[:, dsl])

                    hya = hat_pool.tile([72, RR * SDST], bf16, tag="hya")
                    hx = hat_pool.tile([72, SS * SDST], bf16, tag="hx")
                    for i in range(RR):
                        hsl = slice(i * SDST, (i + 1) * SDST)
                        t_abs = dwork.tile([72, SDST], bf16, tag="t_abs")
                        nc.scalar.activation(t_abs[:], oy_s[:], AF.Abs,
                                             bias=biasc[0:72, 3 + i:4 + i])
                        t_hat = dwork.tile([72, SDST], bf16, tag="t_hat")
                        nc.scalar.activation(t_hat[:], t_abs[:], AF.Relu,
                                             bias=biasc[0:72, 8:9], scale=-1.0)
                        nc.vector.tensor_mul(hya[:, hsl], t_hat[:], aw_s[:])
                        t_abs2 = dwork.tile([72, SDST], bf16, tag="t_abs2")
                        nc.scalar.activation(t_abs2[:], ox_s[:], AF.Abs,
                                             bias=biasc[0:72, 3 + i:4 + i])
                        nc.scalar.activation(hx[:, hsl], t_abs2[:], AF.Relu,
                                             bias=biasc[0:72, 8:9], scale=-1.0)

                    # M[(g,k), (rr,ss,dst)] = hya_rr (bcast over ss) * hx
                    m_full = mfull_pool.tile([72, MQ], bf16, tag="m_full")
                    hxap = hx[:]
                    for i in range(RR):
                        hb = hya[:, i * SDST:(i + 1) * SDST]
                        hya_b = bass.AP(hb.tensor, hb.offset,
                                        [list(hb.ap[0]), [0, SS], [1, SDST]])
                        hx_b = bass.AP(hxap.tensor, hxap.offset,
                                       [list(hxap.ap[0]), [SDST, SS], [1, SDST]])
                        mo = m_full[:, i * SS * SDST:(i + 1) * SS * SDST]
                        mob = bass.AP(mo.tensor, mo.offset,
                                      [list(mo.ap[0]), [SDST, SS], [1, SDST]])
                        nc.vector.tensor_mul(mob, hya_b, hx_b)
                    dma(mscr[:], m_full[:])

                    u = [[u_pool.tile([128, SDST], bf16, tag=f"u{q}_{k}",
                                      name=f"u{q}_{k}")
                          for k in range(KK)] for q in range(2)]

                    for k in range(KK):
                        ki, kj = k // 3, k % 3
                        mrep = rep_pool.tile([128, MQ], bf16, tag="mrep")
                        rsrc = bass.AP(mscr[:].tensor, k * MQ,
                                       [[KK * MQ, 8], [0, 16], [1, MQ]])
                        dma(mrep[:], rsrc)
                        for q in range(2):
                            ps_u = dpsum.tile([128, SDST], f32, tag="ps_u")
                            for i in range(RR):
                                tmp = rep_pool.tile([128, SS * SDST], bf16,
                                                    tag="tmp")
                                for par in range(2):
                                    sslist = [ss for ss in range(-2, 3)
                                              if (PAD + kj - 1 + ss) % 2 == par]
                                    j0 = sslist[0] + 2
                                    nss = len(sslist)
                                    anchor = ((h0 + PAD + ki - 1 + i - 2) * Wp
                                              + PAD + kj - 1 + sslist[0])
                                    xsrc = xq[q]
                                    if par == 1:
                                        xsrc = xqo[q]
                                        anchor -= 1
                                    xs = win(xsrc, anchor,
                                             [[2, nss], [Wp, SH], [1, W]])
                                    mt_ = mrep[:]
                                    mslice = bass.AP(
                                        mt_.tensor,
                                        mt_.offset + (i * SS + j0) * SDST,
                                        [[mt_.ap[0][0], 128], [2 * SDST, nss],
                                         [W, SH], [1, W]])
                                    tp = tmp[:]
                                    tslice = bass.AP(
                                        tp.tensor, tp.offset + j0 * SDST,
                                        [[tp.ap[0][0], 128], [2 * SDST, nss],
                                         [W, SH], [1, W]])
                                    nc.vector.tensor_mul(tslice, mslice, xs)
                                for j in range(SS):
                                    nc.tensor.matmul(
                                        ps_u[:], ident[:],
                                        tmp[:, j * SDST:(j + 1) * SDST],
                                        start=(i == 0 and j == 0),
                                        stop=(i == RR - 1 and j == SS - 1))
                            nc.scalar.activation(u[q][k][:], ps_u[:], AF.Copy)

                    for mt in range(2):
                        ps_o = dpsum.tile([128, SDST], f32, tag=f"ps_o{mt}")
                        first = True
                        for q in range(2):
                            for k in range(KK):
                                base = (q * KK + k) * O + mt * 128
                                nc.tensor.matmul(
                                    ps_o[:], wt2[:, base:base + 128],
                                    u[q][k][:],
                                    start=first, stop=(q == 1 and k == KK - 1))
                                first = False
                        osb = dwork.tile([128, SDST], f32, tag=f"osb{mt}", bufs=1)
                        nc.scalar.activation(osb[:], ps_o[:], AF.Identity,
                                             bias=bout[:, mt:mt + 1])
                        dma(out_d[mt * 128:(mt + 1) * 128, dsl], osb[:])

    nc.compile()
    return nc


def _prep_inputs(x, w_off, b_off, w_attn, b_attn, w_out, b_out):
    B = x.shape[0]
    och_y = np.array([(g * KK + k) * 2 + 0 for g in range(G) for k in range(KK)])
    och_x = np.array([(g * KK + k) * 2 + 1 for g in range(G) for k in range(KK)])
    wcat = np.concatenate([w_off[och_y], w_off[och_x], w_attn], 0)  # [216,C,3,3]
    bcat = np.concatenate([b_off[och_y], b_off[och_x], b_attn], 0)

    # input-channel partition layout per half ch: row g*16+c' = channel g*32+ch*16+c'
    chmap = np.zeros((2, 128), np.int64)
    for ch in range(2):
        for g in range(G):
            for cp in range(16):
                chmap[ch, g * 16 + cp] = g * 32 + ch * 16 + cp
    wmat = np.zeros((9, 2, 128, 216), np.float32)
    for dy in range(3):
        for dx in range(3):
            s = dy * 3 + dx
            for ch in range(2):
                wmat[s, ch] = wcat[:, chmap[ch], dy, dx].T
    wmat = np.ascontiguousarray(
        wmat.transpose(2, 0, 1, 3).reshape(128, 9 * 2 * 216)).astype(BF16)

    wt = w_out.reshape(O, G, Cg, KK)
    wt2 = np.zeros((2, KK, 128, O), np.float32)
    for ch in range(2):
        for k in range(KK):
            for g in range(G):
                wt2[ch, k, g * 16:(g + 1) * 16] = \
                    wt[:, g, ch * 16:(ch + 1) * 16, k].T
    wt2 = np.ascontiguousarray(
        wt2.transpose(2, 0, 1, 3).reshape(128, 2 * KK * O)).astype(BF16)

    sel = np.zeros((72, 8), np.float32)
    rep = np.zeros((8, 72), np.float32)
    for g in range(G):
        sel[g * KK:(g + 1) * KK, g] = 1.0
        rep[g, g * KK:(g + 1) * KK] = 1.0

    biasc = np.zeros((128, 9), np.float32)
    biasc[:72, 0] = bcat[0:72]
    biasc[:72, 1] = bcat[72:144]
    biasc[:72, 2] = bcat[144:216]
    for i in range(5):
        biasc[:, 3 + i] = -(i - 2)
    biasc[:, 8] = 1.0
    bout2 = np.zeros((128, 2), np.float32)
    bout2[:, 0] = b_out[0:128]
    bout2[:, 1] = b_out[128:256]
    ident = np.eye(128, dtype=np.float32).astype(BF16)

    per_core = []
    for b in range(B):
        xpad = np.zeros((C, Hp, Wp), np.float32)
        xpad[:, PAD:PAD + H, PAD:PAD + W] = x[b]
        xpad = xpad.reshape(C, NPIX)[chmap.reshape(-1)].reshape(2, 128, NPIX)
        xqo = np.zeros_like(xpad)
        xqo[:, :, :-1] = xpad[:, :, 1:]
        per_core.append({
            "xq": xpad.astype(BF16),
            "xqo": xqo.astype(BF16),
            "wmat": wmat, "wt2": wt2, "sel": sel, "rep": rep,
            "biasc": biasc, "bout": bout2, "ident": ident,
        })
    return per_core


def kernel(x, w_off, b_off, w_attn, b_attn, w_out, b_out):
    from concourse.bass_utils import run_bass_kernel_spmd

    in_maps = _prep_inputs(np.asarray(x, np.float32),
                           np.asarray(w_off, np.float32),
                           np.asarray(b_off, np.float32),
                           np.asarray(w_attn, np.float32),
                           np.asarray(b_attn, np.float32),
                           np.asarray(w_out, np.float32),
                           np.asarray(b_out, np.float32))
    if "nc" not in _COMPILED:
        _COMPILED["nc"] = _build_kernel()
    nc = _COMPILED["nc"]
    res = run_bass_kernel_spmd(nc, in_maps, list(range(8)))
    out = np.stack([r["out"].reshape(O, H, W) for r in res.results], 0)
    return out.astype(np.float32)



# revision 2
# speedup vs baseline: 1.0152x; 1.0152x over previous
"""Trainium2 Bass kernel for modulated deformable attention (deform_conv2d v2).

Sharding: data-parallel over batch B=8, one image per NeuronCore.

v4 (gather architecture, software-pipelined): exact 4-corner bilinear
sampling via per-16-partition-group indexed gathers (gpsimd ap_gather +
some on DVE indirect_copy), replacing the dense 5x5 shift window:
  - offset/attn convs on PE as 9 shifted-AP matmuls accumulating in PSUM,
    fused per-stripe (no DRAM spill of conv outputs).
  - floor(off) via round-nearest int16 cast (conv bias pre-shifted +3.5);
    corner indices idx = F4y*72 + F4x + static base map, wrapped into the
    per-core (16-partition) layout with one DRAM round trip.
  - per (tap, half): gather 4 corner values per pixel from the fp32
    padded image; multiply by replicated corner weights attn*wy*wx
    (bf16) on DVE (some via an ACT fp32->bf16 cast to balance engines);
    4 identity matmuls accumulate the corners in PSUM.
  - final contraction over (g,c,k)=2304 on PE in 18 accumulation chunks.
  - stripe s+1's conv/index/weight build is interleaved into stripe s's
    tap loop so Pool (gathers) never idles at stripe boundaries.
"""
import numpy as np
import ml_dtypes

G, KK, Kk = 8, 9, 3
Cg, C, O = 32, 256, 256
H = W = 64
HW = H * W
PAD = 4
Hp = Wp = H + 2 * PAD  # 72
NPIX = Hp * Wp  # 5184
NSTRIPE = 8
SH = H // NSTRIPE  # 8 dst rows per stripe
SDST = SH * W  # 512 dst pixels per stripe
SLAB_ROWS = 14
SLAB = SLAB_ROWS * Wp  # 1008 gather source elems per tap slab
XROWS = 16  # fp32 slab rows resident per stripe
XSLAB = XROWS * Wp  # 1152

# engine assignment: per (k,q) unit index 0..17
ACT_CAST_UNITS = frozenset({0, 1, 2, 3, 4, 5, 6, 9, 10, 11, 12, 13})

BF16 = ml_dtypes.bfloat16

_COMPILED = {}


def _build_kernel():
    import concourse.bass as bass
    import concourse.bacc as bacc
    import concourse.tile as tile
    import concourse.mybir as mybir

    f32 = mybir.dt.float32
    bf16 = mybir.dt.bfloat16
    i16 = mybir.dt.int16
    u16 = mybir.dt.uint16
    AF = mybir.ActivationFunctionType
    ALU = mybir.AluOpType

    nc = bacc.Bacc("TRN2", target_bir_lowering=False, num_devices=8)

    xbf_d = nc.dram_tensor("xbf", [2, 128, NPIX], bf16, kind="ExternalInput")
    xf_d = nc.dram_tensor("xf", [2, 128, NPIX], f32, kind="ExternalInput")
    wmat_d = nc.dram_tensor("wmat", [128, 9 * 2 * 216], bf16, kind="ExternalInput")
    wt2_d = nc.dram_tensor("wt2", [128, 2 * KK * O], bf16, kind="ExternalInput")
    sel_d = nc.dram_tensor("sel", [72, 8], f32, kind="ExternalInput")
    rep_d = nc.dram_tensor("rep", [8, 72], f32, kind="ExternalInput")
    bias_d = nc.dram_tensor("biasc", [128, 4], f32, kind="ExternalInput")
    bmap_d = nc.dram_tensor("bmap", [72, SDST], f32, kind="ExternalInput")
    offtab_d = nc.dram_tensor("offtab", [128, 4 * 32], i16,
                              kind="ExternalInput")
    bout_d = nc.dram_tensor("bout", [128, 2], f32, kind="ExternalInput")
    ident_d = nc.dram_tensor("ident", [128, 128], bf16, kind="ExternalInput")
    out_d = nc.dram_tensor("out", [O, HW], f32, kind="ExternalOutput")
    # DRAM scratch (double-buffered across stripes)
    idx_scr = [nc.dram_tensor(f"idxscr{i}", [72, SDST], i16) for i in range(2)]
    w4_scr = [nc.dram_tensor(f"w4scr{i}", [72, 4 * SDST], bf16)
              for i in range(2)]

    def win(t, anchor, dims):
        ap = t[:]
        return bass.AP(ap.tensor, ap.offset + anchor,
                       [[ap.ap[0][0], ap.ap[0][1]]] + [list(d) for d in dims])

    with tile.TileContext(nc) as tc:
        with tc.tile_pool(name="io", bufs=1) as io_pool:
            dma = nc.sync.dma_start

            wmat = io_pool.tile([128, 9 * 2 * 216], bf16)
            dma(wmat[:], wmat_d[:])
            wt2 = io_pool.tile([128, 2 * KK * O], bf16)
            sel = io_pool.tile([72, 8], f32)
            rep = io_pool.tile([8, 72], f32)
            biasc = io_pool.tile([128, 4], f32)
            bmap = io_pool.tile([72, SDST], f32)
            offtab = io_pool.tile([128, 4 * 32], i16)
            bout = io_pool.tile([128, 2], f32)
            ident = io_pool.tile([128, 128], bf16)

            def wmat_ap(s, q, m0, m1):
                base = (s * 2 + q) * 216
                return wmat[:, base + m0: base + m1]

            with (
                tc.tile_pool(name="cpsum", bufs=1,
                             space=bass.MemorySpace.PSUM) as cpsum,
                tc.tile_pool(name="cwork", bufs=1) as cwork,
                tc.tile_pool(name="xslab", bufs=2) as xslab,
                tc.tile_pool(name="idxp", bufs=2) as idxp,
                tc.tile_pool(name="rep4", bufs=3) as rep4,
                tc.tile_pool(name="gath", bufs=5) as gpool,
                tc.tile_pool(name="tmp", bufs=4) as tpool,
                tc.tile_pool(name="u", bufs=3) as u_pool,
                tc.tile_pool(name="dpsum", bufs=2,
                             space=bass.MemorySpace.PSUM) as dpsum,
                tc.tile_pool(name="opsum", bufs=1,
                             space=bass.MemorySpace.PSUM) as opsum,
            ):
                state = {}

                def build_conv_mm(st, chunk):
                    """conv matmul chunk (3 of 9 shifts) for stripe st."""
                    r0 = st * SH
                    if chunk == 0:
                        # bf16 conv slab: padded rows [r0+3, r0+13]
                        xbs = [xslab.tile([128, 11 * Wp], bf16, tag=f"xbs{q}",
                                          name=f"xbs{q}") for q in range(2)]
                        for q in range(2):
                            dma(xbs[q][:],
                                xbf_d[q][:, (r0 + 3) * Wp:(r0 + 14) * Wp])
                        # fp32 gather slabs for this stripe
                        xfs = [xslab.tile([128, XSLAB], f32, tag=f"xfs{q}",
                                          name=f"xfs{q}") for q in range(2)]
                        for q in range(2):
                            dma(xfs[q][:],
                                xf_d[q][:, r0 * Wp:r0 * Wp + XSLAB])
                        state[st] = {
                            "xbs": xbs, "xfs": xfs,
                            "ps_y": cpsum.tile([72, SDST], f32, tag="ps_y",
                                               name="ps_y"),
                            "ps_x": cpsum.tile([72, SDST], f32, tag="ps_x",
                                               name="ps_x"),
                            "ps_a": cpsum.tile([72, SDST], f32, tag="ps_a",
                                               name="ps_a"),
                        }
                    sd = state[st]
                    xbs = sd["xbs"]
                    ps_y, ps_x, ps_a = sd["ps_y"], sd["ps_x"], sd["ps_a"]
                    for s in range(chunk * 3, chunk * 3 + 3):
                        dy, dx = s // 3, s % 3
                        for q in range(2):
                            anchor = dy * Wp + (PAD + dx - 1)
                            rhs = win(xbs[q], anchor, [[Wp, SH], [1, W]])
                            first = (s == 0) and (q == 0)
                            last = (s == 8) and (q == 1)
                            nc.tensor.matmul(ps_y[:], wmat_ap(s, q, 0, 72),
                                             rhs, start=first, stop=last)
                            nc.tensor.matmul(ps_x[:], wmat_ap(s, q, 72, 144),
                                             rhs, start=first, stop=last)
                            nc.tensor.matmul(ps_a[:], wmat_ap(s, q, 144, 216),
                                             rhs, start=first, stop=last)

                def build_conv_post(st):
                    """conv evac + attn softmax for stripe st."""
                    sd = state[st]
                    ps_y, ps_x, ps_a = sd["ps_y"], sd["ps_x"], sd["ps_a"]
                    oy4 = cwork.tile([72, SDST], f32, tag="oy4")
                    ox4 = cwork.tile([72, SDST], f32, tag="ox4")
                    nc.scalar.activation(oy4[:], ps_y[:], AF.Identity,
                                         bias=biasc[0:72, 0:1])
                    nc.scalar.activation(ox4[:], ps_x[:], AF.Identity,
                                         bias=biasc[0:72, 1:2])
                    att_e = cwork.tile([72, SDST], f32, tag="att_e")
                    nc.scalar.activation(att_e[:], ps_a[:], AF.Exp,
                                         bias=biasc[0:72, 2:3])
                    ps_sr = cpsum.tile([72, SDST], f32, tag="ps_sr")
                    nc.tensor.matmul(ps_sr[0:8, :], sel[:], att_e[:],
                                     start=True, stop=True)
                    rcp = cwork.tile([8, SDST], f32, tag="rcp")
                    nc.vector.reciprocal(rcp[:], ps_sr[0:8, :])
                    nc.tensor.matmul(ps_sr[:], rep[:], rcp[:],
                                     start=True, stop=True)
                    awt = cwork.tile([72, SDST], bf16, tag="awt")
                    nc.vector.tensor_mul(awt[:], att_e[:], ps_sr[:])
                    # fp32 gather slabs for this stripe
                    sd["oy4"], sd["ox4"], sd["awt"] = oy4, ox4, awt

                def build_floors(st):
                    """floor(off) for stripe st (feeds both idx and weights)."""
                    sd = state[st]
                    oy4, ox4 = sd["oy4"], sd["ox4"]
                    f4y_i = cwork.tile([72, SDST], i16, tag="f4y_i")
                    f4x_i = cwork.tile([72, SDST], i16, tag="f4x_i")
                    nc.vector.tensor_copy(f4y_i[:], oy4[:])
                    nc.vector.tensor_copy(f4x_i[:], ox4[:])
                    f4y_r = cwork.tile([72, SDST], f32, tag="f4y_r")
                    f4x_r = cwork.tile([72, SDST], f32, tag="f4x_r")
                    nc.vector.tensor_copy(f4y_r[:], f4y_i[:])
                    nc.vector.tensor_copy(f4x_r[:], f4x_i[:])
                    f4y = cwork.tile([72, SDST], f32, tag="f4y")
                    f4x = cwork.tile([72, SDST], f32, tag="f4x")
                    nc.vector.tensor_scalar(f4y[:], f4y_r[:], 6.0, 1.0,
                                            ALU.min, ALU.max)
                    nc.vector.tensor_scalar(f4x[:], f4x_r[:], 6.0, 1.0,
                                            ALU.min, ALU.max)
                    sd["f4y"], sd["f4x"] = f4y, f4x

                def build_w4(st):
                    """bilinear corner weights for stripe st."""
                    sd = state[st]
                    oy4, ox4, awt = sd["oy4"], sd["ox4"], sd["awt"]
                    f4y, f4x = sd["f4y"], sd["f4x"]
                    dyt = cwork.tile([72, SDST], f32, tag="dyt")
                    dxt = cwork.tile([72, SDST], f32, tag="dxt")
                    nc.vector.tensor_sub(dyt[:], oy4[:], f4y[:])
                    nc.vector.tensor_sub(dxt[:], ox4[:], f4x[:])
                    # wy/wx on ACT: w1 = d + .5 ; w0 = -d + .5
                    wy1 = cwork.tile([72, SDST], f32, tag="wy1")
                    wy0 = cwork.tile([72, SDST], f32, tag="wy0")
                    wx1 = cwork.tile([72, SDST], f32, tag="wx1")
                    wx0 = cwork.tile([72, SDST], f32, tag="wx0")
                    half = biasc[0:72, 3:4]
                    nc.scalar.activation(wy1[:], dyt[:], AF.Identity,
                                         bias=half)
                    nc.scalar.activation(wy0[:], dyt[:], AF.Identity,
                                         bias=half, scale=-1.0)
                    nc.scalar.activation(wx1[:], dxt[:], AF.Identity,
                                         bias=half)
                    nc.scalar.activation(wx0[:], dxt[:], AF.Identity,
                                         bias=half, scale=-1.0)
                    ay0 = cwork.tile([72, SDST], bf16, tag="ay0")
                    ay1 = cwork.tile([72, SDST], bf16, tag="ay1")
                    nc.vector.tensor_mul(ay0[:], awt[:], wy0[:])
                    nc.vector.tensor_mul(ay1[:], awt[:], wy1[:])
                    w4 = cwork.tile([72, 4 * SDST], bf16, tag="w4")
                    nc.vector.tensor_mul(w4[:, 0:SDST], ay0[:], wx0[:])
                    nc.vector.tensor_mul(w4[:, SDST:2 * SDST], ay0[:], wx1[:])
                    nc.vector.tensor_mul(w4[:, 2 * SDST:3 * SDST],
                                         ay1[:], wx0[:])
                    nc.vector.tensor_mul(w4[:, 3 * SDST:4 * SDST],
                                         ay1[:], wx1[:])
                    sd["w4"] = w4

                def build_w4_dma(st):
                    dma(w4_scr[st % 2][:], state[st]["w4"][:])

                def build_idx_a(st):
                    """corner index map + wrap DMAs for stripe st."""
                    sd = state[st]
                    f4y, f4x = sd["f4y"], sd["f4x"]
                    idxa = cwork.tile([72, SDST], f32, tag="idxa")
                    nc.vector.tensor_scalar_mul(idxa[:], f4y[:], 72.0)
                    idxb = cwork.tile([72, SDST], f32, tag="idxb")
                    nc.vector.tensor_add(idxb[:], idxa[:], f4x[:])
                    idxf = cwork.tile([72, SDST], f32, tag="idxf")
                    nc.vector.tensor_add(idxf[:], idxb[:], bmap[:])
                    idx16 = cwork.tile([72, SDST], i16, tag="idx16")
                    nc.vector.tensor_copy(idx16[:], idxf[:])
                    sd["idx16"] = idx16

                def build_idx_dma(st):
                    sd = state[st]
                    dma(idx_scr[st % 2][:], sd["idx16"][:])
                    # wrap tap 0 first, then 1-4, then 5-8, so gathers can
                    # start without waiting for the full wrap transfer
                    idxw0 = idxp.tile([128, 32], i16, tag="idxw0")
                    idxwm = idxp.tile([128, 4 * 32], i16, tag="idxwm")
                    idxwr = idxp.tile([128, 4 * 32], i16, tag="idxwr")
                    for g in range(8):
                        base = g * KK * SDST
                        src0 = bass.AP(idx_scr[st % 2][:].tensor, base,
                                       [[1, 16], [16, 32]])
                        dma(idxw0[g * 16:(g + 1) * 16, :], src0)
                    for g in range(8):
                        base = g * KK * SDST + SDST
                        srcm = bass.AP(idx_scr[st % 2][:].tensor, base,
                                       [[1, 16], [16, 4 * 32]])
                        dma(idxwm[g * 16:(g + 1) * 16, :], srcm)
                    for g in range(8):
                        base = g * KK * SDST + 5 * SDST
                        srcr = bass.AP(idx_scr[st % 2][:].tensor, base,
                                       [[1, 16], [16, 4 * 32]])
                        dma(idxwr[g * 16:(g + 1) * 16, :], srcr)
                    sd["idxw0"], sd["idxwm"], sd["idxwr"] = idxw0, idxwm, idxwr

                def _expand(idxw_ap, nk, out_tag):
                    idxc = idxp.tile([128, nk * 4 * 32], i16, tag=out_tag,
                                     name=out_tag)
                    iwb = bass.AP(idxw_ap.tensor, idxw_ap.offset,
                                  [list(idxw_ap.ap[0]), [32, nk], [0, 4],
                                   [1, 32]])
                    ot = offtab[:]
                    otb = bass.AP(ot.tensor, ot.offset,
                                  [list(ot.ap[0]), [0, nk], [32, 4], [1, 32]])
                    ic = idxc[:]
                    icb = bass.AP(ic.tensor, ic.offset,
                                  [list(ic.ap[0]), [128, nk], [32, 4],
                                   [1, 32]])
                    nc.vector.tensor_add(icb, iwb, otb)
                    return idxc

                def build_idx_b0(st):
                    sd = state[st]
                    sd["idxc0"] = _expand(sd["idxw0"][:], 1, "idxc0")

                def build_idx_bm(st):
                    sd = state[st]
                    sd["idxcm"] = _expand(sd["idxwm"][:], 4, "idxcm")

                def build_idx_br(st):
                    sd = state[st]
                    sd["idxcr"] = _expand(sd["idxwr"][:], 4, "idxcr")

                def idxc_slice(sd, k):
                    if k == 0:
                        return sd["idxc0"][:, 0:128]
                    if k <= 4:
                        return sd["idxcm"][:, (k - 1) * 128:k * 128]
                    return sd["idxcr"][:, (k - 5) * 128:(k - 4) * 128]

                def fetch_w4rep(st, k):
                    # issued from the ACT queue: its dep (w4_scr) is always
                    # ready, so it never head-of-line-blocks, and it stays
                    # clear of the SP queue's dependent build DMAs.
                    sd = state[st]
                    w4rep = rep4.tile([128, 4 * SDST], bf16, tag="w4rep")
                    rsrc = bass.AP(w4_scr[st % 2][:].tensor,
                                   k * 4 * SDST,
                                   [[KK * 4 * SDST, 8], [0, 16],
                                    [1, 4 * SDST]])
                    dma(w4rep[:], rsrc)
                    sd.setdefault("w4q", []).append(w4rep)

                def unit(st, uix, _unused):
                    """one (k, q) gather/modulate/accumulate unit."""
                    sd = state[st]
                    k, q = uix // 2, uix % 2
                    ki = k // 3
                    if q == 0 and k + 1 < KK:
                        fetch_w4rep(st, k + 1)  # prefetch next tap's weights
                    w4rep = sd["w4q"][k]
                    slab = sd["xfs"][q][:, ki * Wp:ki * Wp + SLAB]
                    gath = gpool.tile([128, 4 * SDST], f32, tag="gath")
                    nc.gpsimd.ap_gather(
                        gath[:], slab, idxc_slice(sd, k),
                        channels=128, num_elems=SLAB, d=1,
                        num_idxs=4 * SDST)
                    tmp = tpool.tile([128, 4 * SDST], bf16, tag="tmp")
                    if uix in ACT_CAST_UNITS:
                        gcast = tpool.tile([128, 4 * SDST], bf16, tag="gcast")
                        nc.scalar.activation(gcast[:], gath[:], AF.Copy)
                        nc.vector.tensor_mul(tmp[:], gcast[:], w4rep[:])
                    else:
                        nc.vector.tensor_mul(tmp[:], gath[:], w4rep[:])
                    ps_u = dpsum.tile([128, SDST], f32, tag="ps_u")
                    for c in range(4):
                        nc.tensor.matmul(ps_u[:], ident[:],
                                         tmp[:, c * SDST:(c + 1) * SDST],
                                         start=(c == 0), stop=(c == 3))
                    ut = u_pool.tile([128, SDST], bf16, tag="ut")
                    nc.scalar.activation(ut[:], ps_u[:], AF.Copy)
                    sd.setdefault("pend", []).append((ut, q, k))

                def flush_final(st, n):
                    """issue up to n pending final-contraction matmul pairs."""
                    sd = state[st]
                    pend = sd.get("pend", [])
                    for _ in range(n):
                        if not pend:
                            return
                        ut, q, k = pend.pop(0)
                        nflushed = sd.get("nflushed", 0)
                        for mt in range(2):
                            base = (q * KK + k) * O + mt * 128
                            nc.tensor.matmul(sd["ps_o"][mt][:],
                                             wt2[:, base:base + 128], ut[:],
                                             start=(nflushed == 0),
                                             stop=(nflushed == 17))
                        sd["nflushed"] = nflushed + 1

                def final(st):
                    sd = state[st]
                    dsl = slice(st * SDST, (st + 1) * SDST)
                    for mt in range(2):
                        osb = cwork.tile([128, SDST], f32, tag=f"osb{mt}")
                        nc.scalar.activation(osb[:], sd["ps_o"][mt][:],
                                             AF.Identity,
                                             bias=bout[:, mt:mt + 1])
                        dma(out_d[mt * 128:(mt + 1) * 128, dsl], osb[:])

                # ---- software-pipelined stripe loop ----
                def build_all(st):
                    build_conv_mm(st, 0)
                    for t_, d_ in ((sel, sel_d), (rep, rep_d),
                                   (biasc, bias_d), (bmap, bmap_d),
                                   (offtab, offtab_d)):
                        dma(t_[:], d_[:])
                    for c in range(1, 3):
                        build_conv_mm(st, c)
                    build_conv_post(st)
                    build_floors(st)
                    build_idx_a(st)
                    build_idx_dma(st)
                    build_idx_b0(st)
                    build_idx_bm(st)
                    build_idx_br(st)
                    build_w4(st)
                    build_w4_dma(st)
                    fetch_w4rep(st, 0)

                build_all(0)
                # non-critical loads: queued behind stripe 0's build DMAs
                dma(wt2[:], wt2_d[:])
                dma(ident[:], ident_d[:])
                dma(bout[:], bout_d[:])
                for st in range(NSTRIPE):
                    state[st]["ps_o"] = [
                        opsum.tile([128, SDST], f32, tag=f"ps_o{mt}",
                                   name=f"ps_o{mt}")
                        for mt in range(2)]
                    nxt = st + 1
                    for uix in range(18):
                        unit(st, uix, None)
                        if uix >= 2:
                            flush_final(st, 1)  # lag-2 final matmuls
                        if nxt < NSTRIPE:
                            if uix == 2:
                                build_conv_mm(nxt, 0)
                            elif uix == 4:
                                build_conv_mm(nxt, 1)
                            elif uix == 6:
                                build_conv_mm(nxt, 2)
                            elif uix == 7:
                                build_conv_post(nxt)
                            elif uix == 8:
                                build_floors(nxt)
                            elif uix == 9:
                                build_idx_a(nxt)
                                build_w4(nxt)
                            elif uix == 10:
                                build_idx_dma(nxt)
                            elif uix == 11:
                                build_idx_b0(nxt)
                            elif uix == 12:
                                build_idx_bm(nxt)
                                build_w4_dma(nxt)
                            elif uix == 14:
                                build_idx_br(nxt)
                            elif uix == 13:
                                fetch_w4rep(nxt, 0)
                    flush_final(st, 18)
                    final(st)
                    if st - 1 >= 0:
                        state.pop(st - 1, None)

    nc.compile()
    return nc


def _prep_inputs(x, w_off, b_off, w_attn, b_attn, w_out, b_out):
    B = x.shape[0]
    och_y = np.array([(g * KK + k) * 2 + 0 for g in range(G) for k in range(KK)])
    och_x = np.array([(g * KK + k) * 2 + 1 for g in range(G) for k in range(KK)])
    wcat = np.concatenate([w_off[och_y], w_off[och_x], w_attn], 0)  # [216,C,3,3]
    bcat = np.concatenate([b_off[och_y], b_off[och_x], b_attn], 0)

    chmap = np.zeros((2, 128), np.int64)
    for ch in range(2):
        for g in range(G):
            for cp in range(16):
                chmap[ch, g * 16 + cp] = g * 32 + ch * 16 + cp
    wmat = np.zeros((9, 2, 128, 216), np.float32)
    for dy in range(3):
        for dx in range(3):
            s = dy * 3 + dx
            for ch in range(2):
                wmat[s, ch] = wcat[:, chmap[ch], dy, dx].T
    wmat = np.ascontiguousarray(
        wmat.transpose(2, 0, 1, 3).reshape(128, 9 * 2 * 216)).astype(BF16)

    wt = w_out.reshape(O, G, Cg, KK)
    wt2 = np.zeros((2, KK, 128, O), np.float32)
    for ch in range(2):
        for k in range(KK):
            for g in range(G):
                wt2[ch, k, g * 16:(g + 1) * 16] = \
                    wt[:, g, ch * 16:(ch + 1) * 16, k].T
    wt2 = np.ascontiguousarray(
        wt2.transpose(2, 0, 1, 3).reshape(128, 2 * KK * O)).astype(BF16)

    sel = np.zeros((72, 8), np.float32)
    rep = np.zeros((8, 72), np.float32)
    for g in range(G):
        sel[g * KK:(g + 1) * KK, g] = 1.0
        rep[g, g * KK:(g + 1) * KK] = 1.0

    biasc = np.zeros((128, 4), np.float32)
    biasc[:72, 0] = bcat[0:72] + 3.5
    biasc[:72, 1] = bcat[72:144] + 3.5
    biasc[:72, 2] = bcat[144:216]
    biasc[:, 3] = 0.5

    # static base map: B[(g,k),(r',cl)] = (r'-1)*72 + cl + kj - 1
    bmap = np.zeros((72, SDST), np.float32)
    rr, cl = np.meshgrid(np.arange(SH), np.arange(W), indexing="ij")
    for g in range(G):
        for k in range(KK):
            kj = k % 3
            bmap[g * KK + k] = ((rr - 1) * Wp + cl + kj - 1).reshape(-1)

    offtab = np.zeros((128, 4, 32), np.int16)
    corner = np.array([0, 1, Wp, Wp + 1], np.int16)
    offtab[:, :, :] = corner[None, :, None]
    offtab = offtab.reshape(128, 4 * 32)

    bout2 = np.zeros((128, 2), np.float32)
    bout2[:, 0] = b_out[0:128]
    bout2[:, 1] = b_out[128:256]
    ident = np.eye(128, dtype=np.float32).astype(BF16)

    per_core = []
    for b in range(B):
        xpad = np.zeros((C, Hp, Wp), np.float32)
        xpad[:, PAD:PAD + H, PAD:PAD + W] = x[b]
        xpad = xpad.reshape(C, NPIX)[chmap.reshape(-1)].reshape(2, 128, NPIX)
        per_core.append({
            "xbf": xpad.astype(BF16),
            "xf": xpad.astype(np.float32),
            "wmat": wmat, "wt2": wt2, "sel": sel, "rep": rep,
            "biasc": biasc, "bmap": bmap, "offtab": offtab,
            "bout": bout2, "ident": ident,
        })
    return per_core


def kernel(x, w_off, b_off, w_attn, b_attn, w_out, b_out):
    from concourse.bass_utils import run_bass_kernel_spmd

    in_maps = _prep_inputs(np.asarray(x, np.float32),
                           np.asarray(w_off, np.float32),
                           np.asarray(b_off, np.float32),
                           np.asarray(w_attn, np.float32),
                           np.asarray(b_attn, np.float32),
                           np.asarray(w_out, np.float32),
                           np.asarray(b_out, np.float32))
    if "nc" not in _COMPILED:
        _COMPILED["nc"] = _build_kernel()
    nc = _COMPILED["nc"]
    res = run_bass_kernel_spmd(nc, in_maps, list(range(8)))
    out = np.stack([r["out"].reshape(O, H, W) for r in res.results], 0)
    return out.astype(np.float32)


# revision 3
# speedup vs baseline: 1.0277x; 1.0123x over previous
"""Trainium2 Bass kernel for modulated deformable attention (deform_conv2d v2).

Sharding: data-parallel over batch B=8, one image per NeuronCore.

v4 (gather architecture, software-pipelined): exact 4-corner bilinear
sampling via per-16-partition-group indexed gathers (gpsimd ap_gather +
some on DVE indirect_copy), replacing the dense 5x5 shift window:
  - offset/attn convs on PE as 9 shifted-AP matmuls accumulating in PSUM,
    fused per-stripe (no DRAM spill of conv outputs).
  - floor(off) via round-nearest int16 cast (conv bias pre-shifted +3.5);
    corner indices idx = F4y*72 + F4x + static base map, wrapped into the
    per-core (16-partition) layout with one DRAM round trip.
  - per (tap, half): gather 4 corner values per pixel from the fp32
    padded image; multiply by replicated corner weights attn*wy*wx
    (bf16) on DVE (some via an ACT fp32->bf16 cast to balance engines);
    4 identity matmuls accumulate the corners in PSUM.
  - final contraction over (g,c,k)=2304 on PE in 18 accumulation chunks.
  - stripe s+1's conv/index/weight build is interleaved into stripe s's
    tap loop so Pool (gathers) never idles at stripe boundaries.
"""
import numpy as np
import ml_dtypes

G, KK, Kk = 8, 9, 3
Cg, C, O = 32, 256, 256
H = W = 64
HW = H * W
PAD = 4
Hp = Wp = H + 2 * PAD  # 72
NPIX = Hp * Wp  # 5184
NSTRIPE = 8
SH = H // NSTRIPE  # 8 dst rows per stripe
SDST = SH * W  # 512 dst pixels per stripe
SLAB_ROWS = 14
SLAB = SLAB_ROWS * Wp  # 1008 gather source elems per tap slab
XROWS = 16  # fp32 slab rows resident per stripe
XSLAB = XROWS * Wp  # 1152

# engine assignment: per (k,q) unit index 0..17
ACT_CAST_UNITS = frozenset({0, 1, 2, 3, 4, 5, 6, 9, 10, 11, 12, 13})

BF16 = ml_dtypes.bfloat16

_COMPILED = {}


def _build_kernel():
    import concourse.bass as bass
    import concourse.bacc as bacc
    import concourse.tile as tile
    import concourse.mybir as mybir

    f32 = mybir.dt.float32
    bf16 = mybir.dt.bfloat16
    i16 = mybir.dt.int16
    u16 = mybir.dt.uint16
    AF = mybir.ActivationFunctionType
    ALU = mybir.AluOpType

    nc = bacc.Bacc("TRN2", target_bir_lowering=False, num_devices=8)

    xbf_d = nc.dram_tensor("xbf", [2, 128, NPIX], bf16, kind="ExternalInput")
    xf_d = nc.dram_tensor("xf", [2, 128, NPIX], f32, kind="ExternalInput")
    wmat_d = nc.dram_tensor("wmat", [128, 9 * 2 * 216], bf16, kind="ExternalInput")
    wt2_d = nc.dram_tensor("wt2", [128, 2 * KK * O], bf16, kind="ExternalInput")
    sel_d = nc.dram_tensor("sel", [72, 8], f32, kind="ExternalInput")
    rep_d = nc.dram_tensor("rep", [8, 72], f32, kind="ExternalInput")
    bias_d = nc.dram_tensor("biasc", [128, 4], f32, kind="ExternalInput")
    bmap_d = nc.dram_tensor("bmap", [72, SDST], f32, kind="ExternalInput")
    offtab_d = nc.dram_tensor("offtab", [128, 4 * 32], i16,
                              kind="ExternalInput")
    bout_d = nc.dram_tensor("bout", [128, 2], f32, kind="ExternalInput")
    ident_d = nc.dram_tensor("ident", [128, 128], bf16, kind="ExternalInput")
    out_d = nc.dram_tensor("out", [O, HW], f32, kind="ExternalOutput")
    # DRAM scratch (double-buffered across stripes)
    idx_scr = [nc.dram_tensor(f"idxscr{i}", [72, SDST], i16) for i in range(2)]
    w4_scr = [nc.dram_tensor(f"w4scr{i}", [72, 4 * SDST], bf16)
              for i in range(2)]

    def win(t, anchor, dims):
        ap = t[:]
        return bass.AP(ap.tensor, ap.offset + anchor,
                       [[ap.ap[0][0], ap.ap[0][1]]] + [list(d) for d in dims])

    with tile.TileContext(nc) as tc:
        with tc.tile_pool(name="io", bufs=1) as io_pool:
            dma = nc.sync.dma_start

            wmat = io_pool.tile([128, 9 * 2 * 216], bf16)
            dma(wmat[:, 0:6 * 216], wmat_d[:, 0:6 * 216])
            dma(wmat[:, 6 * 216:], wmat_d[:, 6 * 216:])
            wt2 = io_pool.tile([128, 2 * KK * O], bf16)
            sel = io_pool.tile([72, 8], f32)
            rep = io_pool.tile([8, 72], f32)
            biasc = io_pool.tile([128, 4], f32)
            bmap = io_pool.tile([72, SDST], f32)
            offtab = io_pool.tile([128, 4 * 32], i16)
            bout = io_pool.tile([128, 2], f32)
            ident = io_pool.tile([128, 128], bf16)

            def wmat_ap(s, q, m0, m1):
                base = (s * 2 + q) * 216
                return wmat[:, base + m0: base + m1]

            with (
                tc.tile_pool(name="cpsum", bufs=1,
                             space=bass.MemorySpace.PSUM) as cpsum,
                tc.tile_pool(name="cwork", bufs=1) as cwork,
                tc.tile_pool(name="xslab", bufs=2) as xslab,
                tc.tile_pool(name="idxp", bufs=2) as idxp,
                tc.tile_pool(name="rep4", bufs=4) as rep4,
                tc.tile_pool(name="gath", bufs=6) as gpool,
                tc.tile_pool(name="tmp", bufs=4) as tpool,
                tc.tile_pool(name="u", bufs=4) as u_pool,
                tc.tile_pool(name="dpsum", bufs=2,
                             space=bass.MemorySpace.PSUM) as dpsum,
                tc.tile_pool(name="opsum", bufs=1,
                             space=bass.MemorySpace.PSUM) as opsum,
            ):
                state = {}

                def build_conv_mm(st, chunk):
                    """conv matmul chunk (3 of 9 shifts) for stripe st."""
                    r0 = st * SH
                    if chunk == 0:
                        # bf16 conv slab: padded rows [r0+3, r0+13]
                        xbs = [xslab.tile([128, 11 * Wp], bf16, tag=f"xbs{q}",
                                          name=f"xbs{q}") for q in range(2)]
                        for q in range(2):
                            dma(xbs[q][:],
                                xbf_d[q][:, (r0 + 3) * Wp:(r0 + 14) * Wp])
                        # fp32 gather slabs for this stripe
                        xfs = [xslab.tile([128, XSLAB], f32, tag=f"xfs{q}",
                                          name=f"xfs{q}") for q in range(2)]
                        for q in range(2):
                            dma(xfs[q][:],
                                xf_d[q][:, r0 * Wp:r0 * Wp + XSLAB])
                        state[st] = {
                            "xbs": xbs, "xfs": xfs,
                            "ps_y": cpsum.tile([72, SDST], f32, tag="ps_y",
                                               name="ps_y"),
                            "ps_x": cpsum.tile([72, SDST], f32, tag="ps_x",
                                               name="ps_x"),
                            "ps_a": cpsum.tile([72, SDST], f32, tag="ps_a",
                                               name="ps_a"),
                        }
                    sd = state[st]
                    xbs = sd["xbs"]
                    ps_y, ps_x, ps_a = sd["ps_y"], sd["ps_x"], sd["ps_a"]
                    for s in range(chunk * 3, chunk * 3 + 3):
                        dy, dx = s // 3, s % 3
                        for q in range(2):
                            anchor = dy * Wp + (PAD + dx - 1)
                            rhs = win(xbs[q], anchor, [[Wp, SH], [1, W]])
                            first = (s == 0) and (q == 0)
                            last = (s == 8) and (q == 1)
                            nc.tensor.matmul(ps_y[:], wmat_ap(s, q, 0, 72),
                                             rhs, start=first, stop=last)
                            nc.tensor.matmul(ps_x[:], wmat_ap(s, q, 72, 144),
                                             rhs, start=first, stop=last)
                            nc.tensor.matmul(ps_a[:], wmat_ap(s, q, 144, 216),
                                             rhs, start=first, stop=last)

                def build_conv_post(st):
                    """conv evac + attn softmax for stripe st."""
                    sd = state[st]
                    ps_y, ps_x, ps_a = sd["ps_y"], sd["ps_x"], sd["ps_a"]
                    oy4 = cwork.tile([72, SDST], f32, tag="oy4")
                    ox4 = cwork.tile([72, SDST], f32, tag="ox4")
                    nc.scalar.activation(oy4[:], ps_y[:], AF.Identity,
                                         bias=biasc[0:72, 0:1])
                    nc.scalar.activation(ox4[:], ps_x[:], AF.Identity,
                                         bias=biasc[0:72, 1:2])
                    att_e = cwork.tile([72, SDST], f32, tag="att_e")
                    nc.scalar.activation(att_e[:], ps_a[:], AF.Exp,
                                         bias=biasc[0:72, 2:3])
                    ps_sr = cpsum.tile([72, SDST], f32, tag="ps_sr")
                    nc.tensor.matmul(ps_sr[0:8, :], sel[:], att_e[:],
                                     start=True, stop=True)
                    rcp = cwork.tile([8, SDST], f32, tag="rcp")
                    nc.vector.reciprocal(rcp[:], ps_sr[0:8, :])
                    nc.tensor.matmul(ps_sr[:], rep[:], rcp[:],
                                     start=True, stop=True)
                    awt = cwork.tile([72, SDST], bf16, tag="awt")
                    nc.vector.tensor_mul(awt[:], att_e[:], ps_sr[:])
                    # fp32 gather slabs for this stripe
                    sd["oy4"], sd["ox4"], sd["awt"] = oy4, ox4, awt

                def build_floors(st):
                    """floor(off) for stripe st (feeds both idx and weights)."""
                    sd = state[st]
                    oy4, ox4 = sd["oy4"], sd["ox4"]
                    f4y_i = cwork.tile([72, SDST], i16, tag="f4y_i")
                    f4x_i = cwork.tile([72, SDST], i16, tag="f4x_i")
                    nc.vector.tensor_copy(f4y_i[:], oy4[:])
                    nc.vector.tensor_copy(f4x_i[:], ox4[:])
                    f4y_r = cwork.tile([72, SDST], f32, tag="f4y_r")
                    f4x_r = cwork.tile([72, SDST], f32, tag="f4x_r")
                    nc.vector.tensor_copy(f4y_r[:], f4y_i[:])
                    nc.vector.tensor_copy(f4x_r[:], f4x_i[:])
                    f4y = cwork.tile([72, SDST], f32, tag="f4y")
                    f4x = cwork.tile([72, SDST], f32, tag="f4x")
                    nc.vector.tensor_scalar(f4y[:], f4y_r[:], 6.0, 1.0,
                                            ALU.min, ALU.max)
                    nc.vector.tensor_scalar(f4x[:], f4x_r[:], 6.0, 1.0,
                                            ALU.min, ALU.max)
                    sd["f4y"], sd["f4x"] = f4y, f4x

                def build_w4(st):
                    """bilinear corner weights for stripe st."""
                    sd = state[st]
                    oy4, ox4, awt = sd["oy4"], sd["ox4"], sd["awt"]
                    f4y, f4x = sd["f4y"], sd["f4x"]
                    dyt = cwork.tile([72, SDST], f32, tag="dyt")
                    dxt = cwork.tile([72, SDST], f32, tag="dxt")
                    nc.vector.tensor_sub(dyt[:], oy4[:], f4y[:])
                    nc.vector.tensor_sub(dxt[:], ox4[:], f4x[:])
                    # wy/wx on ACT: w1 = d + .5 ; w0 = -d + .5
                    wy1 = cwork.tile([72, SDST], f32, tag="wy1")
                    wy0 = cwork.tile([72, SDST], f32, tag="wy0")
                    wx1 = cwork.tile([72, SDST], f32, tag="wx1")
                    wx0 = cwork.tile([72, SDST], f32, tag="wx0")
                    half = biasc[0:72, 3:4]
                    nc.scalar.activation(wy1[:], dyt[:], AF.Identity,
                                         bias=half)
                    nc.scalar.activation(wy0[:], dyt[:], AF.Identity,
                                         bias=half, scale=-1.0)
                    nc.scalar.activation(wx1[:], dxt[:], AF.Identity,
                                         bias=half)
                    nc.scalar.activation(wx0[:], dxt[:], AF.Identity,
                                         bias=half, scale=-1.0)
                    ay0 = cwork.tile([72, SDST], bf16, tag="ay0")
                    ay1 = cwork.tile([72, SDST], bf16, tag="ay1")
                    nc.vector.tensor_mul(ay0[:], awt[:], wy0[:])
                    nc.vector.tensor_mul(ay1[:], awt[:], wy1[:])
                    w4 = cwork.tile([72, 4 * SDST], bf16, tag="w4")
                    nc.vector.tensor_mul(w4[:, 0:SDST], ay0[:], wx0[:])
                    nc.vector.tensor_mul(w4[:, SDST:2 * SDST], ay0[:], wx1[:])
                    nc.vector.tensor_mul(w4[:, 2 * SDST:3 * SDST],
                                         ay1[:], wx0[:])
                    nc.vector.tensor_mul(w4[:, 3 * SDST:4 * SDST],
                                         ay1[:], wx1[:])
                    sd["w4"] = w4

                def build_w4_dma(st):
                    dma(w4_scr[st % 2][:], state[st]["w4"][:])

                def build_idx_a(st):
                    """corner index map + wrap DMAs for stripe st."""
                    sd = state[st]
                    f4y, f4x = sd["f4y"], sd["f4x"]
                    idxa = cwork.tile([72, SDST], f32, tag="idxa")
                    nc.vector.tensor_scalar_mul(idxa[:], f4y[:], 72.0)
                    idxb = cwork.tile([72, SDST], f32, tag="idxb")
                    nc.vector.tensor_add(idxb[:], idxa[:], f4x[:])
                    idxf = cwork.tile([72, SDST], f32, tag="idxf")
                    nc.vector.tensor_add(idxf[:], idxb[:], bmap[:])
                    idx16 = cwork.tile([72, SDST], i16, tag="idx16")
                    nc.vector.tensor_copy(idx16[:], idxf[:])
                    sd["idx16"] = idx16

                def build_idx_dma(st):
                    sd = state[st]
                    dma(idx_scr[st % 2][:], sd["idx16"][:])
                    # wrap tap 0 first, then 1-4, then 5-8, so gathers can
                    # start without waiting for the full wrap transfer
                    idxw0 = idxp.tile([128, 32], i16, tag="idxw0")
                    idxwm = idxp.tile([128, 4 * 32], i16, tag="idxwm")
                    idxwr = idxp.tile([128, 4 * 32], i16, tag="idxwr")
                    for g in range(8):
                        base = g * KK * SDST
                        src0 = bass.AP(idx_scr[st % 2][:].tensor, base,
                                       [[1, 16], [16, 32]])
                        dma(idxw0[g * 16:(g + 1) * 16, :], src0)
                    for g in range(8):
                        base = g * KK * SDST + SDST
                        srcm = bass.AP(idx_scr[st % 2][:].tensor, base,
                                       [[1, 16], [16, 4 * 32]])
                        dma(idxwm[g * 16:(g + 1) * 16, :], srcm)
                    for g in range(8):
                        base = g * KK * SDST + 5 * SDST
                        srcr = bass.AP(idx_scr[st % 2][:].tensor, base,
                                       [[1, 16], [16, 4 * 32]])
                        dma(idxwr[g * 16:(g + 1) * 16, :], srcr)
                    sd["idxw0"], sd["idxwm"], sd["idxwr"] = idxw0, idxwm, idxwr

                def _expand(idxw_ap, nk, out_tag):
                    idxc = idxp.tile([128, nk * 4 * 32], i16, tag=out_tag,
                                     name=out_tag)
                    iwb = bass.AP(idxw_ap.tensor, idxw_ap.offset,
                                  [list(idxw_ap.ap[0]), [32, nk], [0, 4],
                                   [1, 32]])
                    ot = offtab[:]
                    otb = bass.AP(ot.tensor, ot.offset,
                                  [list(ot.ap[0]), [0, nk], [32, 4], [1, 32]])
                    ic = idxc[:]
                    icb = bass.AP(ic.tensor, ic.offset,
                                  [list(ic.ap[0]), [128, nk], [32, 4],
                                   [1, 32]])
                    nc.vector.tensor_add(icb, iwb, otb)
                    return idxc

                def build_idx_b0(st):
                    sd = state[st]
                    sd["idxc0"] = _expand(sd["idxw0"][:], 1, "idxc0")

                def build_idx_bm(st):
                    sd = state[st]
                    sd["idxcm"] = _expand(sd["idxwm"][:], 4, "idxcm")

                def build_idx_br(st):
                    sd = state[st]
                    sd["idxcr"] = _expand(sd["idxwr"][:], 4, "idxcr")

                def idxc_slice(sd, k):
                    if k == 0:
                        return sd["idxc0"][:, 0:128]
                    if k <= 4:
                        return sd["idxcm"][:, (k - 1) * 128:k * 128]
                    return sd["idxcr"][:, (k - 5) * 128:(k - 4) * 128]

                def fetch_w4rep(st, k):
                    # issued from the ACT queue: its dep (w4_scr) is always
                    # ready, so it never head-of-line-blocks, and it stays
                    # clear of the SP queue's dependent build DMAs.
                    sd = state[st]
                    w4rep = rep4.tile([128, 4 * SDST], bf16, tag="w4rep")
                    rsrc = bass.AP(w4_scr[st % 2][:].tensor,
                                   k * 4 * SDST,
                                   [[KK * 4 * SDST, 8], [0, 16],
                                    [1, 4 * SDST]])
                    dma(w4rep[:], rsrc)
                    sd.setdefault("w4q", []).append(w4rep)

                def unit(st, uix, _unused):
                    """one (k, q) gather/modulate/accumulate unit."""
                    sd = state[st]
                    k, q = uix // 2, uix % 2
                    ki = k // 3
                    if q == 0 and k + 1 < KK:
                        fetch_w4rep(st, k + 1)  # prefetch next tap's weights
                    w4rep = sd["w4q"][k]
                    slab = sd["xfs"][q][:, ki * Wp:ki * Wp + SLAB]
                    gath = gpool.tile([128, 4 * SDST], f32, tag="gath")
                    nc.gpsimd.ap_gather(
                        gath[:], slab, idxc_slice(sd, k),
                        channels=128, num_elems=SLAB, d=1,
                        num_idxs=4 * SDST)
                    tmp = tpool.tile([128, 4 * SDST], bf16, tag="tmp")
                    if uix in ACT_CAST_UNITS:
                        gcast = tpool.tile([128, 4 * SDST], bf16, tag="gcast")
                        nc.scalar.activation(gcast[:], gath[:], AF.Copy)
                        nc.vector.tensor_mul(tmp[:], gcast[:], w4rep[:])
                    else:
                        nc.vector.tensor_mul(tmp[:], gath[:], w4rep[:])
                    ps_u = dpsum.tile([128, SDST], f32, tag="ps_u")
                    for c in range(4):
                        nc.tensor.matmul(ps_u[:], ident[:],
                                         tmp[:, c * SDST:(c + 1) * SDST],
                                         start=(c == 0), stop=(c == 3))
                    ut = u_pool.tile([128, SDST], bf16, tag="ut")
                    nc.scalar.activation(ut[:], ps_u[:], AF.Copy)
                    sd.setdefault("pend", []).append((ut, q, k))

                def flush_final(st, n):
                    """issue up to n pending final-contraction matmul pairs."""
                    sd = state[st]
                    pend = sd.get("pend", [])
                    for _ in range(n):
                        if not pend:
                            return
                        ut, q, k = pend.pop(0)
                        nflushed = sd.get("nflushed", 0)
                        for mt in range(2):
                            base = (q * KK + k) * O + mt * 128
                            nc.tensor.matmul(sd["ps_o"][mt][:],
                                             wt2[:, base:base + 128], ut[:],
                                             start=(nflushed == 0),
                                             stop=(nflushed == 17))
                        sd["nflushed"] = nflushed + 1

                def final(st):
                    sd = state[st]
                    dsl = slice(st * SDST, (st + 1) * SDST)
                    for mt in range(2):
                        osb = cwork.tile([128, SDST], f32, tag=f"osb{mt}")
                        nc.scalar.activation(osb[:], sd["ps_o"][mt][:],
                                             AF.Identity,
                                             bias=bout[:, mt:mt + 1])
                        dma(out_d[mt * 128:(mt + 1) * 128, dsl], osb[:])

                # ---- software-pipelined stripe loop ----
                def build_all(st):
                    build_conv_mm(st, 0)
                    # preload the ACT function table off the critical path
                    dummy = cwork.tile([128, 8], f32, tag="dummy")
                    nc.vector.memset(dummy[:], 0.0)
                    nc.scalar.activation(dummy[:], dummy[:], AF.Exp, bias=0.0)
                    for t_, d_ in ((sel, sel_d), (rep, rep_d),
                                   (biasc, bias_d), (bmap, bmap_d),
                                   (offtab, offtab_d)):
                        dma(t_[:], d_[:])
                    for c in range(1, 3):
                        build_conv_mm(st, c)
                    build_conv_post(st)
                    build_floors(st)
                    build_idx_a(st)
                    build_idx_dma(st)
                    build_idx_b0(st)
                    build_idx_bm(st)
                    build_idx_br(st)
                    build_w4(st)
                    build_w4_dma(st)
                    fetch_w4rep(st, 0)

                build_all(0)
                # non-critical loads: queued behind stripe 0's build DMAs
                dma(wt2[:], wt2_d[:])
                dma(ident[:], ident_d[:])
                dma(bout[:], bout_d[:])
                for st in range(NSTRIPE):
                    state[st]["ps_o"] = [
                        opsum.tile([128, SDST], f32, tag=f"ps_o{mt}",
                                   name=f"ps_o{mt}")
                        for mt in range(2)]
                    nxt = st + 1
                    for uix in range(18):
                        unit(st, uix, None)
                        if uix >= 2:
                            flush_final(st, 1)  # lag-2 final matmuls
                        if nxt < NSTRIPE:
                            if uix == 2:
                                build_conv_mm(nxt, 0)
                            elif uix == 4:
                                build_conv_mm(nxt, 1)
                            elif uix == 6:
                                build_conv_mm(nxt, 2)
                            elif uix == 7:
                                build_conv_post(nxt)
                            elif uix == 8:
                                build_floors(nxt)
                            elif uix == 9:
                                build_idx_a(nxt)
                                build_w4(nxt)
                            elif uix == 10:
                                build_idx_dma(nxt)
                            elif uix == 11:
                                build_idx_b0(nxt)
                            elif uix == 12:
                                build_idx_bm(nxt)
                                build_w4_dma(nxt)
                            elif uix == 14:
                                build_idx_br(nxt)
                            elif uix == 13:
                                fetch_w4rep(nxt, 0)
                    flush_final(st, 18)
                    final(st)
                    if st - 1 >= 0:
                        state.pop(st - 1, None)

    nc.compile()
    return nc


def _prep_inputs(x, w_off, b_off, w_attn, b_attn, w_out, b_out):
    B = x.shape[0]
    och_y = np.array([(g * KK + k) * 2 + 0 for g in range(G) for k in range(KK)])
    och_x = np.array([(g * KK + k) * 2 + 1 for g in range(G) for k in range(KK)])
    wcat = np.concatenate([w_off[och_y], w_off[och_x], w_attn], 0)  # [216,C,3,3]
    bcat = np.concatenate([b_off[och_y], b_off[och_x], b_attn], 0)

    chmap = np.zeros((2, 128), np.int64)
    for ch in range(2):
        for g in range(G):
            for cp in range(16):
                chmap[ch, g * 16 + cp] = g * 32 + ch * 16 + cp
    wmat = np.zeros((9, 2, 128, 216), np.float32)
    for dy in range(3):
        for dx in range(3):
            s = dy * 3 + dx
            for ch in range(2):
                wmat[s, ch] = wcat[:, chmap[ch], dy, dx].T
    wmat = np.ascontiguousarray(
        wmat.transpose(2, 0, 1, 3).reshape(128, 9 * 2 * 216)).astype(BF16)

    wt = w_out.reshape(O, G, Cg, KK)
    wt2 = np.zeros((2, KK, 128, O), np.float32)
    for ch in range(2):
        for k in range(KK):
            for g in range(G):
                wt2[ch, k, g * 16:(g + 1) * 16] = \
                    wt[:, g, ch * 16:(ch + 1) * 16, k].T
    wt2 = np.ascontiguousarray(
        wt2.transpose(2, 0, 1, 3).reshape(128, 2 * KK * O)).astype(BF16)

    sel = np.zeros((72, 8), np.float32)
    rep = np.zeros((8, 72), np.float32)
    for g in range(G):
        sel[g * KK:(g + 1) * KK, g] = 1.0
        rep[g, g * KK:(g + 1) * KK] = 1.0

    biasc = np.zeros((128, 4), np.float32)
    biasc[:72, 0] = bcat[0:72] + 3.5
    biasc[:72, 1] = bcat[72:144] + 3.5
    biasc[:72, 2] = bcat[144:216]
    biasc[:, 3] = 0.5

    # static base map: B[(g,k),(r',cl)] = (r'-1)*72 + cl + kj - 1
    bmap = np.zeros((72, SDST), np.float32)
    rr, cl = np.meshgrid(np.arange(SH), np.arange(W), indexing="ij")
    for g in range(G):
        for k in range(KK):
            kj = k % 3
            bmap[g * KK + k] = ((rr - 1) * Wp + cl + kj - 1).reshape(-1)

    offtab = np.zeros((128, 4, 32), np.int16)
    corner = np.array([0, 1, Wp, Wp + 1], np.int16)
    offtab[:, :, :] = corner[None, :, None]
    offtab = offtab.reshape(128, 4 * 32)

    bout2 = np.zeros((128, 2), np.float32)
    bout2[:, 0] = b_out[0:128]
    bout2[:, 1] = b_out[128:256]
    ident = np.eye(128, dtype=np.float32).astype(BF16)

    per_core = []
    for b in range(B):
        xpad = np.zeros((C, Hp, Wp), np.float32)
        xpad[:, PAD:PAD + H, PAD:PAD + W] = x[b]
        xpad = xpad.reshape(C, NPIX)[chmap.reshape(-1)].reshape(2, 128, NPIX)
        per_core.append({
            "xbf": xpad.astype(BF16),
            "xf": xpad.astype(np.float32),
            "wmat": wmat, "wt2": wt2, "sel": sel, "rep": rep,
            "biasc": biasc, "bmap": bmap, "offtab": offtab,
            "bout": bout2, "ident": ident,
        })
    return per_core


def kernel(x, w_off, b_off, w_attn, b_attn, w_out, b_out):
    from concourse.bass_utils import run_bass_kernel_spmd

    in_maps = _prep_inputs(np.asarray(x, np.float32),
                           np.asarray(w_off, np.float32),
                           np.asarray(b_off, np.float32),
                           np.asarray(w_attn, np.float32),
                           np.asarray(b_attn, np.float32),
                           np.asarray(w_out, np.float32),
                           np.asarray(b_out, np.float32))
    if "nc" not in _COMPILED:
        _COMPILED["nc"] = _build_kernel()
    nc = _COMPILED["nc"]
    res = run_bass_kernel_spmd(nc, in_maps, list(range(8)))
    out = np.stack([r["out"].reshape(O, H, W) for r in res.results], 0)
    return out.astype(np.float32)
